# revision 14
# baseline (speedup 1.0000x reference)
"""ClusterAttention (segment_reduce) Trainium2 kernel.

Strategy: shard by cluster ("bucket"). The host groups point indices by
bucket (argsort of cluster_ids — pure index preprocessing), pads each
bucket's point list to a multiple of 16 (a "slot") by duplicating the
bucket's first point, and cuts the bucket list into 8 contiguous,
point-balanced core shards. Every core therefore owns whole buckets and
needs NO cross-core communication:

  pass 1: x -> transpose -> k^T,v^T matmuls; per-slot sums of x (PE one-hot
          matmuls, pad-masked) and per-slot maxes of k^T,v^T (DVE reduce).
  mid:    slot->bucket combine (gpsimd ap_gather + DVE reduce), then build
          per-bucket context tables using the linearity of segment-sum
          (k_sum[b] = x_sum[b] @ Wk + cnt*bk):
            tbl[...,0] = scale*(x_mean@Wk + bk)        (k_mean, pre-scaled)
            tbl[...,1] = scale*(max_k + bk)            (k_max,  pre-scaled)
            tbl[...,2] = [v_mean|v_max] @ Wvc + bvc    (v_combined)
          and expand per-slot context (ap_gather).
  pass 2: x^T (host-pretransposed) -> q^T; interaction = (q^T+bq)*ctx
          (slot-broadcast); gate MLP; out^T = Wp.T @ (gate*v_comb) + bp.

The output is returned transposed+permuted; the host scatters rows back
(duplicate pad rows rewrite identical values).
"""

import numpy as np

import concourse.bass as bass
from concourse import bacc
import concourse.mybir as mybir
import concourse.tile as tile
import concourse.bass_utils as bass_utils
from concourse.masks import make_identity

# problem constants (hardcoded per contract)
N_GLOBAL = 500000
C = 128
B_GLOBAL = 8192
NUM_HEADS = 4
NCORES = 8
SLOT = 16          # points per slot (pad unit)
CHUNK = 512        # points per device chunk
TPC = CHUNK // 128  # 128-row tiles per chunk
SCALE = float((C // NUM_HEADS) ** (-0.5))
NEG_BIG = -1.0e30

f32 = mybir.dt.float32
i16 = mybir.dt.int16
X = mybir.AxisListType.X
ALU = mybir.AluOpType
ACTF = mybir.ActivationFunctionType


def _wrap16(vals):
    """ap_gather index layout: [128, n//16] int16, idx j read from
    partition j%16 (replicated across the 8 gpsimd cores' groups)."""
    v = np.asarray(vals, np.int16)
    n = v.size
    assert n % 16 == 0
    arr = np.zeros((128, n // 16), np.int16)
    k = np.arange(n)
    for g in range(8):
        arr[16 * g + (k % 16), k // 16] = v
    return arr


def _build_layout(ids, B, ncores):
    """Bucket-grouped, slot-padded permutation + all per-core metadata."""
    N = ids.shape[0]
    counts = np.bincount(ids, minlength=B).astype(np.int64)
    order = np.argsort(ids, kind="stable")
    starts = np.zeros(B + 1, np.int64)
    starts[1:] = np.cumsum(counts)
    nslots_b = (counts + SLOT - 1) // SLOT
    padded_b = nslots_b * SLOT

    cum = np.cumsum(padded_b)
    total = cum[-1]
    cuts = [0]
    for c in range(1, ncores):
        cuts.append(int(np.searchsorted(cum, c * total / ncores)))
    cuts.append(B)

    core_npts = [int(padded_b[cuts[c]:cuts[c + 1]].sum()) for c in range(ncores)]
    core_nbux = [cuts[c + 1] - cuts[c] for c in range(ncores)]
    NPTS = max(CHUNK, int(-(-max(core_npts) // CHUNK)) * CHUNK)
    NBUX = max(16, int(-(-max(core_nbux) // 16)) * 16)
    NSLOT = NPTS // SLOT
    SPB = max(1, int(nslots_b.max()))  # max slots per bucket
    NTILE = NPTS // 128

    cores = []
    for c in range(ncores):
        bs, be = cuts[c], cuts[c + 1]
        nb = be - bs
        cnts = counts[bs:be]
        nsl = nslots_b[bs:be]
        sstart = np.zeros(nb + 1, np.int64)
        sstart[1:] = np.cumsum(nsl)
        tot_slots = int(sstart[-1])
        dst0 = sstart[:-1] * SLOT

        perm = np.full(NPTS, -1, np.int64)
        is_real = np.zeros(NPTS, bool)
        src = order[starts[bs]:starts[be]]
        if src.size:
            seg0 = (starts[bs:be] - starts[bs])
            pos = np.repeat(dst0, cnts) + (np.arange(src.size) - np.repeat(seg0, cnts))
            perm[pos] = src
            is_real[pos] = True
        padcnt = (nsl * SLOT - cnts)
        if padcnt.sum():
            off = np.concatenate([[0], np.cumsum(padcnt)])[:-1]
            ppos = np.repeat(dst0 + cnts, padcnt) + (np.arange(int(padcnt.sum())) - np.repeat(off, padcnt))
            first = np.where(cnts > 0, order[starts[bs:be]], 0)
            perm[ppos] = np.repeat(first, padcnt)

        si = np.zeros((NPTS, 8), np.float32)
        p_idx = np.arange(NPTS) % 128
        si[np.arange(NPTS), p_idx // 16] = is_real.astype(np.float32)
        slot_ind = si.reshape(NTILE, 128, 8)

        cnt_rep = np.ones(NBUX, np.float32)
        cnt_rep[:nb] = np.maximum(cnts, 1).astype(np.float32)
        cnt_rep = np.broadcast_to(cnt_rep, (128, NBUX)).copy()

        jj = np.arange(SPB)[None, :]
        bmat = np.where(jj < nsl[:, None], sstart[:-1][:, None] + jj, NSLOT)
        L = np.full(NBUX * SPB, NSLOT, np.int64)
        L[: nb * SPB] = bmat.ravel()
        bidx = _wrap16(L)

        S = np.full(NSLOT, NBUX, np.int64)
        if tot_slots:
            S[:tot_slots] = np.repeat(np.arange(nb), nsl)
        sidx = _wrap16(S)

        cores.append(dict(perm=perm, slot_ind=slot_ind, cnt=cnt_rep,
                          bidx=bidx, sidx=sidx))
    return dict(NPTS=NPTS, NBUX=NBUX, NSLOT=NSLOT, SPB=SPB, NTILE=NTILE,
                NCHUNK=NPTS // CHUNK, cores=cores)


def _build_nc(L):
    NPTS, NBUX, NSLOT, SPB = L["NPTS"], L["NBUX"], L["NSLOT"], L["SPB"]
    NTILE, NCHUNK = L["NTILE"], L["NCHUNK"]

    nc = bacc.Bacc("TRN2", target_bir_lowering=False)
    xp = nc.dram_tensor("xp", [NPTS, C], f32, kind="ExternalInput")
    xtp = nc.dram_tensor("xtp", [C, NPTS], f32, kind="ExternalInput")
    sind_d = nc.dram_tensor("sind", [NTILE, 128, 8], f32, kind="ExternalInput")
    cnt_d = nc.dram_tensor("cnt", [128, NBUX], f32, kind="ExternalInput")
    bidx_d = nc.dram_tensor("bidx", [128, (NBUX * SPB) // 16], i16, kind="ExternalInput")
    sidx_d = nc.dram_tensor("sidx", [128, NSLOT // 16], i16, kind="ExternalInput")
    w_d = {}
    for nm, sh in [("Wq", [C, C]), ("Wk", [C, C]), ("Wv", [C, C]),
                   ("Wg1", [2 * C, C]), ("Wg2", [C, C]), ("Wvc", [2 * C, C]),
                   ("Wp", [C, C])]:
        w_d[nm] = nc.dram_tensor(nm, sh, f32, kind="ExternalInput")
    b_d = {}
    for nm in ["bq", "bk", "bv", "bg1", "bg2", "bvc", "bp"]:
        b_d[nm] = nc.dram_tensor(nm, [C], f32, kind="ExternalInput")
    ot = nc.dram_tensor("ot", [C, NPTS], f32, kind="ExternalOutput")

    xv = xp[:].rearrange("(c t p) f -> c p t f", p=128, t=TPC)

    with tile.TileContext(nc) as tc:
        with tc.tile_pool(name="consts", bufs=1) as consts, \
             tc.tile_pool(name="tables", bufs=1) as tables:
            ident = consts.tile([128, 128], f32)
            make_identity(nc, ident[:])
            w = {}
            for nm in ["Wq", "Wk", "Wv", "Wg2", "Wp"]:
                w[nm] = consts.tile([C, C], f32, name="w_" + nm, tag="w_" + nm)
                nc.sync.dma_start(out=w[nm][:], in_=w_d[nm][:])
            for nm in ["Wg1", "Wvc"]:
                w[nm + "a"] = consts.tile([C, C], f32, name="w_" + nm + "a", tag="w_" + nm + "a")
                w[nm + "b"] = consts.tile([C, C], f32, name="w_" + nm + "b", tag="w_" + nm + "b")
                nc.sync.dma_start(out=w[nm + "a"][:], in_=w_d[nm][0:C, :])
                nc.sync.dma_start(out=w[nm + "b"][:], in_=w_d[nm][C:2 * C, :])
            b = {}
            for nm in b_d:
                b[nm] = consts.tile([C, 1], f32, name="b_" + nm, tag="b_" + nm)
                nc.sync.dma_start(out=b[nm][:], in_=b_d[nm][:, None])
            bk_s = consts.tile([C, 1], f32)
            nc.scalar.activation(out=bk_s[:], in_=b["bk"][:], func=ACTF.Identity,
                                 scale=SCALE)
            # Warm every PE-read constant once so steady-state matmuls carry a
            # single sync wait (walrus can't encode >1 wait on LDWEIGHTS).
            with tc.tile_pool(name="warmps", bufs=1, space="PSUM") as warmps:
                scratch = warmps.tile([128, 128], f32)
                nc.tensor.transpose(out=scratch[:], in_=ident[:], identity=ident[:])
                for wt in ["Wq", "Wk", "Wv", "Wg2", "Wp", "Wg1a", "Wg1b",
                           "Wvca", "Wvcb"]:
                    nc.tensor.matmul(out=scratch[:, 0:1], lhsT=w[wt][:],
                                     rhs=ident[:, 0:1], start=True, stop=True)

            # ---------------- pass 1 ----------------
            with tc.tile_pool(name="slots", bufs=1) as slots, \
                 tc.tile_pool(name="ctx", bufs=1) as ctxp:
                xslot = slots.tile([128, NSLOT + 1], f32)
                kslot = slots.tile([128, NSLOT + 1], f32)
                vslot = slots.tile([128, NSLOT + 1], f32)
                nc.vector.memset(xslot[:, NSLOT:], 0.0)
                nc.vector.memset(kslot[:, NSLOT:], NEG_BIG)
                nc.vector.memset(vslot[:, NSLOT:], NEG_BIG)

                with tc.tile_pool(name="p1w", bufs=3) as p1w, \
                     tc.tile_pool(name="p1ps", bufs=2, space="PSUM") as p1ps, \
                     tc.tile_pool(name="p1xs", bufs=2, space="PSUM") as p1xs:
                    sind_sb = p1w.tile([128, NTILE, 8], f32, bufs=1)
                    nc.sync.dma_start(out=sind_sb[:],
                                      in_=sind_d[:].rearrange("a p s -> p a s"))
                    for ci in range(NCHUNK):
                        s0 = ci * CHUNK
                        xt4 = p1w.tile([128, TPC, 128], f32, tag="xt4")
                        nc.sync.dma_start(out=xt4[:], in_=xv[ci])
                        xT_ps = p1ps.tile([128, CHUNK], f32, tag="xT")
                        for t in range(TPC):
                            nc.tensor.transpose(out=xT_ps[:, t * 128:(t + 1) * 128],
                                                in_=xt4[:, t, :], identity=ident[:])
                        xT_sb = p1w.tile([128, CHUNK], f32, tag="xTs")
                        nc.scalar.copy(out=xT_sb[:], in_=xT_ps[:])
                        kT_ps = p1ps.tile([128, CHUNK], f32, tag="kT")
                        nc.tensor.matmul(out=kT_ps[:], lhsT=w["Wk"][:], rhs=xT_sb[:],
                                         start=True, stop=True)
                        vT_ps = p1ps.tile([128, CHUNK], f32, tag="vT")
                        nc.tensor.matmul(out=vT_ps[:], lhsT=w["Wv"][:], rhs=xT_sb[:],
                                         start=True, stop=True)
                        xs_ps = p1xs.tile([128, 32], f32, tag="xs")
                        for t in range(TPC):
                            nc.tensor.matmul(
                                out=xs_ps[:, t * 8:(t + 1) * 8],
                                lhsT=xt4[:, t, :],
                                rhs=sind_sb[:, ci * TPC + t, :],
                                start=True, stop=True)
                        nc.scalar.copy(out=xslot[:, ci * 32:ci * 32 + 32], in_=xs_ps[:])
                        nc.vector.reduce_max(
                            out=kslot[:, ci * 32:ci * 32 + 32],
                            in_=kT_ps[:].rearrange("p (s e) -> p s e", e=SLOT), axis=X)
                        nc.vector.reduce_max(
                            out=vslot[:, ci * 32:ci * 32 + 32],
                            in_=vT_ps[:].rearrange("p (s e) -> p s e", e=SLOT), axis=X)

                # ---------------- mid phase ----------------
                tbl = ctxp.tile([128, NBUX + 1, 3], f32)
                ctxslot = tables.tile([128, NSLOT, 3], f32)
                with tc.tile_pool(name="midw", bufs=1) as midw, \
                     tc.tile_pool(name="midps", bufs=2, space="PSUM") as midps:
                    bidx_sb = midw.tile([128, (NBUX * SPB) // 16], i16, tag="bidx")
                    nc.sync.dma_start(out=bidx_sb[:], in_=bidx_d[:])
                    sidx_sb = midw.tile([128, NSLOT // 16], i16, tag="sidx")
                    nc.sync.dma_start(out=sidx_sb[:], in_=sidx_d[:])
                    cnt_sb = midw.tile([128, NBUX], f32, tag="cnt")
                    nc.sync.dma_start(out=cnt_sb[:], in_=cnt_d[:])
                    rc = midw.tile([128, NBUX], f32, tag="rc")
                    nc.vector.reciprocal(out=rc[:], in_=cnt_sb[:])

                    BKB = 512  # buckets per gather block
                    red = {}
                    for nm, src_t, op in [("xbsum", xslot, ALU.add),
                                          ("kbmax", kslot, ALU.max),
                                          ("vbmax", vslot, ALU.max)]:
                        red[nm] = midw.tile([128, NBUX], f32, name="red_" + nm, tag="red_" + nm)
                        for j in range(0, NBUX, BKB):
                            e = min(j + BKB, NBUX)
                            nbk = e - j
                            g = midw.tile([128, nbk * SPB], f32, tag="gst", bufs=2)
                            nc.gpsimd.ap_gather(
                                out_ap=g[:], in_ap=src_t[:, :, None],
                                idxs_ap=bidx_sb[:, (j * SPB) // 16:(e * SPB) // 16],
                                channels=128,
                                num_elems=NSLOT + 1, d=1, num_idxs=nbk * SPB)
                            nc.vector.tensor_reduce(
                                out=red[nm][:, j:e],
                                in_=g[:].rearrange("p (b j) -> p b j", j=SPB),
                                axis=X, op=op)

                    xmean = midw.tile([128, NBUX], f32, tag="xmean")
                    nc.vector.tensor_tensor(out=xmean[:], in0=red["xbsum"][:],
                                            in1=rc[:], op=ALU.mult)

                    def mm_big(ps, lhsT, rhs_t):
                        for j in range(0, NBUX, 512):
                            e = min(j + 512, NBUX)
                            nc.tensor.matmul(out=ps[:, j:e], lhsT=lhsT,
                                             rhs=rhs_t[:, j:e], start=True, stop=True)

                    km_ps = midps.tile([128, NBUX], f32, tag="mmp")
                    mm_big(km_ps, w["Wk"][:], xmean)
                    nc.scalar.activation(out=tbl[:, :NBUX, 0], in_=km_ps[:],
                                         func=ACTF.Identity, scale=SCALE, bias=bk_s[:])
                    nc.scalar.activation(out=tbl[:, :NBUX, 1], in_=red["kbmax"][:],
                                         func=ACTF.Identity, scale=SCALE, bias=bk_s[:])

                    vm_ps = midps.tile([128, NBUX], f32, tag="mmp")
                    mm_big(vm_ps, w["Wv"][:], xmean)
                    vmean = midw.tile([128, NBUX], f32, tag="vmean")
                    nc.scalar.activation(out=vmean[:], in_=vm_ps[:],
                                         func=ACTF.Identity, bias=b["bv"][:])
                    vmax = midw.tile([128, NBUX], f32, tag="vmax")
                    nc.scalar.activation(out=vmax[:], in_=red["vbmax"][:],
                                         func=ACTF.Identity, bias=b["bv"][:])
                    vc_ps = midps.tile([128, NBUX], f32, tag="mmp")
                    for j in range(0, NBUX, 512):
                        e = min(j + 512, NBUX)
                        nc.tensor.matmul(out=vc_ps[:, j:e], lhsT=w["Wvca"][:],
                                         rhs=vmean[:, j:e], start=True, stop=False)
                        nc.tensor.matmul(out=vc_ps[:, j:e], lhsT=w["Wvcb"][:],
                                         rhs=vmax[:, j:e], start=False, stop=True)
                    nc.scalar.activation(out=tbl[:, :NBUX, 2], in_=vc_ps[:],
                                         func=ACTF.Identity, bias=b["bvc"][:])
                    nc.vector.memset(tbl[:, NBUX, :], 0.0)

                    nc.gpsimd.ap_gather(
                        out_ap=ctxslot[:], in_ap=tbl[:],
                        idxs_ap=sidx_sb[:], channels=128,
                        num_elems=NBUX + 1, d=3, num_idxs=NSLOT)

            # ---------------- pass 2 ----------------
            with tc.tile_pool(name="p2w", bufs=3) as p2w, \
                 tc.tile_pool(name="p2ps", bufs=2, space="PSUM") as p2ps, \
                 tc.tile_pool(name="p2po", bufs=2, space="PSUM") as p2po:
                for ci in range(NCHUNK):
                    s0 = ci * CHUNK
                    sl0 = ci * 32
                    xT2 = p2w.tile([128, CHUNK], f32, tag="xT2")
                    nc.sync.dma_start(out=xT2[:], in_=xtp[:, s0:s0 + CHUNK])
                    qT_ps = p2ps.tile([128, CHUNK], f32, tag="qT")
                    nc.tensor.matmul(out=qT_ps[:], lhsT=w["Wq"][:], rhs=xT2[:],
                                     start=True, stop=True)
                    ctx = ctxslot[:, sl0:sl0 + 32, :]
                    inter1 = p2w.tile([128, 32, SLOT], f32, tag="i1")
                    inter2 = p2w.tile([128, 32, SLOT], f32, tag="i2")
                    qv = qT_ps[:].rearrange("p (s e) -> p s e", e=SLOT)
                    nc.vector.scalar_tensor_tensor(
                        out=inter1[:], in0=qv, scalar=b["bq"][:],
                        in1=ctx[:, :, 0:1].broadcast_to([128, 32, SLOT]),
                        op0=ALU.add, op1=ALU.mult)
                    nc.vector.scalar_tensor_tensor(
                        out=inter2[:], in0=qv, scalar=b["bq"][:],
                        in1=ctx[:, :, 1:2].broadcast_to([128, 32, SLOT]),
                        op0=ALU.add, op1=ALU.mult)
                    h1_ps = p2ps.tile([128, CHUNK], f32, tag="h1")
                    nc.tensor.matmul(out=h1_ps[:], lhsT=w["Wg1a"][:],
                                     rhs=inter1[:].rearrange("p a b -> p (a b)"),
                                     start=True, stop=False)
                    nc.tensor.matmul(out=h1_ps[:], lhsT=w["Wg1b"][:],
                                     rhs=inter2[:].rearrange("p a b -> p (a b)"),
                                     start=False, stop=True)
                    h1 = p2w.tile([128, CHUNK], f32, tag="h1s")
                    nc.scalar.activation(out=h1[:], in_=h1_ps[:], func=ACTF.Relu,
                                         bias=b["bg1"][:])
                    h2_ps = p2ps.tile([128, CHUNK], f32, tag="h2")
                    nc.tensor.matmul(out=h2_ps[:], lhsT=w["Wg2"][:], rhs=h1[:],
                                     start=True, stop=True)
                    gate = p2w.tile([128, CHUNK], f32, tag="gate")
                    nc.scalar.activation(out=gate[:], in_=h2_ps[:], func=ACTF.Sigmoid,
                                         bias=b["bg2"][:])
                    gv = p2w.tile([128, 32, SLOT], f32, tag="gv")
                    nc.gpsimd.tensor_tensor(
                        out=gv[:], in0=gate[:].rearrange("p (s e) -> p s e", e=SLOT),
                        in1=ctx[:, :, 2:3].broadcast_to([128, 32, SLOT]), op=ALU.mult)
                    oT_ps = p2po.tile([128, CHUNK], f32, tag="oT")
                    nc.tensor.matmul(out=oT_ps[:], lhsT=w["Wp"][:],
                                     rhs=gv[:].rearrange("p a b -> p (a b)"),
                                     start=True, stop=True)
                    oT = p2w.tile([128, CHUNK], f32, tag="oTs")
                    if ci % 2 == 0:
                        nc.vector.tensor_scalar(out=oT[:], in0=oT_ps[:],
                                                scalar1=b["bp"][:], scalar2=None,
                                                op0=ALU.add)
                    else:
                        nc.scalar.activation(out=oT[:], in_=oT_ps[:],
                                             func=ACTF.Identity, bias=b["bp"][:])
                    nc.scalar.dma_start(out=ot[:, s0:s0 + CHUNK], in_=oT[:])
    nc.finalize()
    return nc


def _make_in_maps(inputs, layout):
    shared = {nm: np.ascontiguousarray(inputs[nm], np.float32)
              for nm in ["Wq", "Wk", "Wv", "Wg1", "Wg2", "Wvc", "Wp",
                         "bq", "bk", "bv", "bg1", "bg2", "bvc", "bp"]}
    x = np.ascontiguousarray(inputs["x"], np.float32)
    in_maps = []
    for core in layout["cores"]:
        perm = core["perm"]
        xp = np.zeros((layout["NPTS"], C), np.float32)
        m = perm >= 0
        xp[m] = x[perm[m]]
        in_maps.append(dict(shared, xp=xp, xtp=np.ascontiguousarray(xp.T),
                            sind=core["slot_ind"], cnt=core["cnt"],
                            bidx=core["bidx"], sidx=core["sidx"]))
    return in_maps


def _assemble_out(results, layout, n):
    out = np.empty((n, C), np.float32)
    for core, r in zip(layout["cores"], results):
        perm = core["perm"]
        m = perm >= 0
        out[perm[m]] = r["ot"].T[m]
    return out


def _run(inputs, layout, trace=False):
    nc = _build_nc(layout)
    in_maps = _make_in_maps(inputs, layout)
    res = bass_utils.run_bass_kernel_spmd(
        nc, in_maps, core_ids=list(range(NCORES)), trace=trace)
    out = _assemble_out(res.results, layout, inputs["x"].shape[0])
    return out, res


def kernel(**inputs):
    ids = np.asarray(inputs["cluster_ids"]).astype(np.int64)
    B = int(inputs["total_buckets"])
    layout = _build_layout(ids, B, NCORES)
    out, _ = _run(inputs, layout, trace=False)
    return out


# ---------------------------------------------------------------------------
# pure-numpy emulation of the device program (for logic validation off-HW)
def kernel_emulate(**inputs):
    ids = np.asarray(inputs["cluster_ids"]).astype(np.int64)
    B = int(inputs["total_buckets"])
    L = _build_layout(ids, B, NCORES)
    NPTS, NBUX, NSLOT, SPB = L["NPTS"], L["NBUX"], L["NSLOT"], L["SPB"]
    x = np.asarray(inputs["x"], np.float32)
    W = {k: np.asarray(inputs[k], np.float32) for k in
         ["Wq", "Wk", "Wv", "Wg1", "Wg2", "Wvc", "Wp",
          "bq", "bk", "bv", "bg1", "bg2", "bvc", "bp"]}
    n = x.shape[0]
    out = np.empty((n, C), np.float32)
    for core in L["cores"]:
        perm = core["perm"]
        m = perm >= 0
        xp = np.zeros((NPTS, C), np.float32)
        xp[m] = x[perm[m]]
        sind = core["slot_ind"].reshape(NPTS, 8)
        # pass 1
        kT = (xp @ W["Wk"]).T  # pre-bias
        vT = (xp @ W["Wv"]).T
        xslot = np.zeros((128, NSLOT + 1), np.float32)
        kslot = np.full((128, NSLOT + 1), NEG_BIG, np.float32)
        vslot = np.full((128, NSLOT + 1), NEG_BIG, np.float32)
        # slot sums via indicator (pads zeroed), slot maxes direct
        ind = np.zeros((NPTS, NSLOT), np.float32)
        srow = np.arange(NPTS) // SLOT
        ind[np.arange(NPTS), srow] = sind[np.arange(NPTS), (np.arange(NPTS) % 128) // 16]
        xslot[:, :NSLOT] = xp.T @ ind
        kslot[:, :NSLOT] = kT.reshape(128, NSLOT, SLOT).max(axis=2)
        vslot[:, :NSLOT] = vT.reshape(128, NSLOT, SLOT).max(axis=2)
        # mid
        def unwrap(arr, n):
            outv = np.zeros(n, np.int64)
            k = np.arange(n)
            outv[k] = arr[(k % 16), k // 16]
            return outv
        bidx = unwrap(core["bidx"], NBUX * SPB)
        sidx = unwrap(core["sidx"], NSLOT)
        g = xslot[:, bidx].reshape(128, NBUX, SPB)
        xbsum = g.sum(axis=2)
        kbmax = kslot[:, bidx].reshape(128, NBUX, SPB).max(axis=2)
        vbmax = vslot[:, bidx].reshape(128, NBUX, SPB).max(axis=2)
        rc = 1.0 / core["cnt"]
        xmean = xbsum * rc
        tbl = np.zeros((128, NBUX + 1, 3), np.float32)
        tbl[:, :NBUX, 0] = SCALE * (W["Wk"].T @ xmean + W["bk"][:, None])
        tbl[:, :NBUX, 1] = SCALE * (kbmax + W["bk"][:, None])
        vmean = W["Wv"].T @ xmean + W["bv"][:, None]
        vmax = vbmax + W["bv"][:, None]
        tbl[:, :NBUX, 2] = (W["Wvc"][:C].T @ vmean + W["Wvc"][C:].T @ vmax
                            + W["bvc"][:, None])
        ctxslot = tbl[:, sidx, :]  # [128, NSLOT, 3]
        # pass 2
        qT = (xp @ W["Wq"]).T + W["bq"][:, None]
        ctxe = np.repeat(ctxslot, SLOT, axis=1)  # [128, NPTS, 3]
        inter1 = qT * ctxe[:, :, 0]
        inter2 = qT * ctxe[:, :, 1]
        h1 = np.maximum(W["Wg1"][:C].T @ inter1 + W["Wg1"][C:].T @ inter2
                        + W["bg1"][:, None], 0.0)
        h2 = W["Wg2"].T @ h1 + W["bg2"][:, None]
        gate = 1.0 / (1.0 + np.exp(-h2))
        gv = gate * ctxe[:, :, 2]
        oT = W["Wp"].T @ gv + W["bp"][:, None]
        out[perm[m]] = oT.T[m]
    return out


# revision 28
# speedup vs baseline: 46.9882x; 46.9882x over previous
"""ClusterAttention (segment_reduce) Trainium2 kernel.

Strategy: shard by cluster ("bucket"). The host groups point indices by
bucket (argsort of cluster_ids — pure index preprocessing), pads each
bucket's point list to a multiple of 16 (a "slot") by duplicating the
bucket's first point, and cuts the bucket list into 8 contiguous,
point-balanced core shards. Every core therefore owns whole buckets and
needs NO cross-core communication:

  pass 1: x -> transpose -> k^T,v^T matmuls; per-slot sums of x (PE one-hot
          matmuls, pad-masked) and per-slot maxes of k^T,v^T (DVE reduce).
  mid:    slot->bucket combine (gpsimd ap_gather + DVE reduce), then build
          per-bucket context tables using the linearity of segment-sum
          (k_sum[b] = x_sum[b] @ Wk + cnt*bk):
            tbl[...,0] = scale*(x_mean@Wk + bk)        (k_mean, pre-scaled)
            tbl[...,1] = scale*(max_k + bk)            (k_max,  pre-scaled)
            tbl[...,2] = [v_mean|v_max] @ Wvc + bvc    (v_combined)
          and expand per-slot context (ap_gather).
  pass 2: x^T (host-pretransposed) -> q^T; interaction = (q^T+bq)*ctx
          (slot-broadcast); gate MLP; out^T = Wp.T @ (gate*v_comb) + bp.

The output is returned transposed+permuted; the host scatters rows back
(duplicate pad rows rewrite identical values).
"""

import numpy as np

import concourse.bass as bass
from concourse import bacc
import concourse.mybir as mybir
import concourse.tile as tile
import concourse.bass_utils as bass_utils
from concourse.masks import make_identity

# problem constants (hardcoded per contract)
N_GLOBAL = 500000
C = 128
B_GLOBAL = 8192
NUM_HEADS = 4
NCORES = 8
SLOT = 16          # points per slot (pad unit)
CHUNK = 512        # points per device chunk
TPC = CHUNK // 128  # 128-row tiles per chunk
SCALE = float((C // NUM_HEADS) ** (-0.5))
NEG_BIG = -1.0e30

f32 = mybir.dt.float32
f32r = mybir.dt.float32r
i16 = mybir.dt.int16
X = mybir.AxisListType.X
ALU = mybir.AluOpType
ACTF = mybir.ActivationFunctionType


def _wrap16(vals):
    """ap_gather index layout: [128, n//16] int16, idx j read from
    partition j%16 (replicated across the 8 gpsimd cores' groups)."""
    v = np.asarray(vals, np.int16)
    n = v.size
    assert n % 16 == 0
    arr = np.zeros((128, n // 16), np.int16)
    k = np.arange(n)
    for g in range(8):
        arr[16 * g + (k % 16), k // 16] = v
    return arr


def _build_layout(ids, B, ncores):
    """Bucket-grouped, slot-padded permutation + all per-core metadata."""
    N = ids.shape[0]
    counts = np.bincount(ids, minlength=B).astype(np.int64)
    order = np.argsort(ids, kind="stable")
    starts = np.zeros(B + 1, np.int64)
    starts[1:] = np.cumsum(counts)
    nslots_b = (counts + SLOT - 1) // SLOT
    padded_b = nslots_b * SLOT

    cum = np.cumsum(padded_b)
    total = cum[-1]
    cuts = [0]
    for c in range(1, ncores):
        cuts.append(int(np.searchsorted(cum, c * total / ncores)))
    cuts.append(B)

    core_npts = [int(padded_b[cuts[c]:cuts[c + 1]].sum()) for c in range(ncores)]
    core_nbux = [cuts[c + 1] - cuts[c] for c in range(ncores)]
    NPTS = max(2 * CHUNK, int(-(-max(core_npts) // (2 * CHUNK))) * 2 * CHUNK)
    NBUX = max(16, int(-(-max(core_nbux) // 16)) * 16)
    NSLOT = NPTS // SLOT
    SPB = max(1, int(nslots_b.max()))  # max slots per bucket
    NTILE = NPTS // 128

    cores = []
    for c in range(ncores):
        bs, be = cuts[c], cuts[c + 1]
        nb = be - bs
        cnts = counts[bs:be]
        nsl = nslots_b[bs:be]
        sstart = np.zeros(nb + 1, np.int64)
        sstart[1:] = np.cumsum(nsl)
        tot_slots = int(sstart[-1])
        dst0 = sstart[:-1] * SLOT

        perm = np.full(NPTS, -1, np.int64)
        is_real = np.zeros(NPTS, bool)
        src = order[starts[bs]:starts[be]]
        if src.size:
            seg0 = (starts[bs:be] - starts[bs])
            pos = np.repeat(dst0, cnts) + (np.arange(src.size) - np.repeat(seg0, cnts))
            perm[pos] = src
            is_real[pos] = True
        padcnt = (nsl * SLOT - cnts)
        if padcnt.sum():
            off = np.concatenate([[0], np.cumsum(padcnt)])[:-1]
            ppos = np.repeat(dst0 + cnts, padcnt) + (np.arange(int(padcnt.sum())) - np.repeat(off, padcnt))
            first = np.where(cnts > 0, order[starts[bs:be]], 0)
            perm[ppos] = np.repeat(first, padcnt)

        si = np.zeros((NPTS, 8), np.float32)
        p_idx = np.arange(NPTS) % 128
        si[np.arange(NPTS), p_idx // 16] = is_real.astype(np.float32)
        slot_ind = si.reshape(NTILE, 128, 8)

        cnt_rep = np.ones(NBUX, np.float32)
        cnt_rep[:nb] = np.maximum(cnts, 1).astype(np.float32)
        cnt_rep = np.broadcast_to(cnt_rep, (128, NBUX)).copy()

        jj = np.arange(SPB)[None, :]
        bmat = np.where(jj < nsl[:, None], sstart[:-1][:, None] + jj, NSLOT)
        L = np.full(NBUX * SPB, NSLOT, np.int64)
        L[: nb * SPB] = bmat.ravel()
        bidx = _wrap16(L)

        S = np.full(NSLOT, NBUX, np.int64)
        if tot_slots:
            S[:tot_slots] = np.repeat(np.arange(nb), nsl)
        sidx = _wrap16(S)

        cores.append(dict(perm=perm, slot_ind=slot_ind, cnt=cnt_rep,
                          bidx=bidx, sidx=sidx))
    return dict(NPTS=NPTS, NBUX=NBUX, NSLOT=NSLOT, SPB=SPB, NTILE=NTILE,
                NCHUNK=NPTS // CHUNK, cores=cores)


def _build_nc(L, reps=1):
    NPTS, NBUX, NSLOT, SPB = L["NPTS"], L["NBUX"], L["NSLOT"], L["SPB"]
    NTILE, NCHUNK = L["NTILE"], L["NCHUNK"]

    nc = bacc.Bacc("TRN2", target_bir_lowering=False)
    xp = nc.dram_tensor("xp", [NPTS, C], f32, kind="ExternalInput")
    xtp = nc.dram_tensor("xtp", [C, NPTS], f32r, kind="ExternalInput")
    sind_d = nc.dram_tensor("sind", [NTILE, 128, 8], f32, kind="ExternalInput")
    cnt_d = nc.dram_tensor("cnt", [128, NBUX], f32, kind="ExternalInput")
    bidx_d = nc.dram_tensor("bidx", [128, (NBUX * SPB) // 16], i16, kind="ExternalInput")
    sidx_d = nc.dram_tensor("sidx", [128, NSLOT // 16], i16, kind="ExternalInput")
    w_d = {}
    for nm, sh in [("Wq", [C, C]), ("Wk", [C, C]), ("Wv", [C, C]),
                   ("Wg1", [2 * C, C]), ("Wg2", [C, C]), ("Wvc", [2 * C, C]),
                   ("Wp", [C, C])]:
        w_d[nm] = nc.dram_tensor(nm, sh, f32, kind="ExternalInput")
    b_d = {}
    for nm in ["bq", "bk", "bv", "bg1", "bg2", "bvc", "bp"]:
        b_d[nm] = nc.dram_tensor(nm, [C], f32, kind="ExternalInput")
    ot = nc.dram_tensor("ot", [C, NPTS], f32, kind="ExternalOutput")

    xv2 = xp[:].rearrange("(c t p) f -> c p t f", p=128, t=2 * TPC)

    def _emit(tc):
        with tc.tile_pool(name="consts", bufs=1) as consts, \
             tc.tile_pool(name="tables", bufs=1) as tables:
            ident = consts.tile([128, 128], f32)
            make_identity(nc, ident[:])
            w = {}
            for nm in ["Wq", "Wk", "Wv", "Wg2", "Wp"]:
                w[nm] = consts.tile([C, C], f32r, name="w_" + nm, tag="w_" + nm)
                nc.sync.dma_start(out=w[nm][:], in_=w_d[nm][:].bitcast(f32r))
            for nm, sl in [("Wg1a", 0), ("Wg1b", 1)]:
                w[nm] = consts.tile([C, C], f32r, name="w_" + nm, tag="w_" + nm)
                nc.sync.dma_start(out=w[nm][:],
                                  in_=w_d["Wg1"][sl * C:(sl + 1) * C, :].bitcast(f32r))
            for nm, sl in [("Wvca", 0), ("Wvcb", 1)]:
                w[nm] = consts.tile([C, C], f32, name="w_" + nm, tag="w_" + nm)
                nc.sync.dma_start(out=w[nm][:], in_=w_d["Wvc"][sl * C:(sl + 1) * C, :])
            w32 = {}
            for nm in ["Wk", "Wv"]:
                w32[nm] = consts.tile([C, C], f32, name="w32_" + nm, tag="w32_" + nm)
                nc.sync.dma_start(out=w32[nm][:], in_=w_d[nm][:])
            b = {}
            for nm in b_d:
                b[nm] = consts.tile([C, 1], f32, name="b_" + nm, tag="b_" + nm)
                nc.sync.dma_start(out=b[nm][:], in_=b_d[nm][:, None])
            bk_s = consts.tile([C, 1], f32)
            nc.scalar.activation(out=bk_s[:], in_=b["bk"][:], func=ACTF.Identity,
                                 scale=SCALE)

            # ---------------- pass 1 ----------------
            with tc.tile_pool(name="slots", bufs=1) as slots, \
                 tc.tile_pool(name="ctx", bufs=1) as ctxp:
                xslot = slots.tile([128, NSLOT + 1], f32)
                kvslot = slots.tile([128, 2, NSLOT + 1], f32)
                nc.vector.memset(xslot[:, NSLOT:], 0.0)
                nc.vector.memset(kvslot[:, :, NSLOT:], NEG_BIG)

                with tc.tile_pool(name="p1w", bufs=3) as p1w, \
                     tc.tile_pool(name="p1ps", bufs=2, space="PSUM") as p1ps, \
                     tc.tile_pool(name="p1xs", bufs=2, space="PSUM") as p1xs:
                    sind_sb = p1w.tile([128, NTILE, 8], f32, bufs=1)
                    nc.sync.dma_start(out=sind_sb[:],
                                      in_=sind_d[:].rearrange("a p s -> p a s"))
                    for cj in range(NCHUNK // 2):
                        xt8 = p1w.tile([128, 2 * TPC, 128], f32, tag="xt8")
                        nc.sync.dma_start(out=xt8[:], in_=xv2[cj])
                        for half in range(2):
                            ci = cj * 2 + half
                            xT_ps = p1ps.tile([128, CHUNK], f32, tag="xT")
                            for t in range(TPC):
                                nc.tensor.transpose(
                                    out=xT_ps[:, t * 128:(t + 1) * 128],
                                    in_=xt8[:, half * TPC + t, :], identity=ident[:])
                            xT_sb = p1w.tile([128, CHUNK], f32r, tag="xTs")
                            nc.scalar.copy(out=xT_sb[:], in_=xT_ps[:])
                            kv_ps = p1ps.tile([128, 2, CHUNK], f32, tag="kv")
                            nc.tensor.matmul(out=kv_ps[:, 0, :], lhsT=w["Wk"][:],
                                             rhs=xT_sb[:], start=True, stop=True)
                            nc.tensor.matmul(out=kv_ps[:, 1, :], lhsT=w["Wv"][:],
                                             rhs=xT_sb[:], start=True, stop=True)
                            xs_ps = p1xs.tile([128, 32], f32, tag="xs")
                            for t in range(TPC):
                                nc.tensor.matmul(
                                    out=xs_ps[:, t * 8:(t + 1) * 8],
                                    lhsT=xt8[:, half * TPC + t, :],
                                    rhs=sind_sb[:, ci * TPC + t, :],
                                    start=True, stop=True)
                            nc.scalar.copy(out=xslot[:, ci * 32:ci * 32 + 32],
                                           in_=xs_ps[:])
                            nc.vector.tensor_reduce(
                                out=kvslot[:, :, ci * 32:ci * 32 + 32],
                                in_=kv_ps[:].rearrange("p u (s e) -> p (u s) e",
                                                       e=SLOT),
                                axis=X, op=ALU.max)

                # ---------------- mid phase ----------------
                tbl = ctxp.tile([128, NBUX + 1, 3], f32)
                ctxslot = tables.tile([128, NSLOT, 3], f32)
                with tc.tile_pool(name="midw", bufs=1) as midw, \
                     tc.tile_pool(name="midps", bufs=2, space="PSUM") as midps:
                    bidx_sb = midw.tile([128, (NBUX * SPB) // 16], i16, tag="bidx")
                    nc.sync.dma_start(out=bidx_sb[:], in_=bidx_d[:])
                    sidx_sb = midw.tile([128, NSLOT // 16], i16, tag="sidx")
                    nc.sync.dma_start(out=sidx_sb[:], in_=sidx_d[:])
                    cnt_sb = midw.tile([128, NBUX], f32, tag="cnt")
                    nc.sync.dma_start(out=cnt_sb[:], in_=cnt_d[:])
                    rc = midw.tile([128, NBUX], f32, tag="rc")
                    nc.vector.reciprocal(out=rc[:], in_=cnt_sb[:])

                    BKB = 512  # buckets per gather block
                    red = {}
                    for nm, src_t, op in [("xbsum", xslot[:, :, None], ALU.add),
                                          ("kbmax", kvslot[:, 0, :, None], ALU.max),
                                          ("vbmax", kvslot[:, 1, :, None], ALU.max)]:
                        red[nm] = midw.tile([128, NBUX], f32, name="red_" + nm, tag="red_" + nm)
                        for j in range(0, NBUX, BKB):
                            e = min(j + BKB, NBUX)
                            nbk = e - j
                            g = midw.tile([128, nbk * SPB], f32, tag="gst", bufs=2)
                            nc.gpsimd.ap_gather(
                                out_ap=g[:], in_ap=src_t,
                                idxs_ap=bidx_sb[:, (j * SPB) // 16:(e * SPB) // 16],
                                channels=128,
                                num_elems=NSLOT + 1, d=1, num_idxs=nbk * SPB)
                            nc.vector.tensor_reduce(
                                out=red[nm][:, j:e],
                                in_=g[:].rearrange("p (b j) -> p b j", j=SPB),
                                axis=X, op=op)

                    xmean = midw.tile([128, NBUX], f32, tag="xmean")
                    nc.vector.tensor_tensor(out=xmean[:], in0=red["xbsum"][:],
                                            in1=rc[:], op=ALU.mult)

                    def mm_big(ps, lhsT, rhs_t):
                        for j in range(0, NBUX, 512):
                            e = min(j + 512, NBUX)
                            nc.tensor.matmul(out=ps[:, j:e], lhsT=lhsT,
                                             rhs=rhs_t[:, j:e], start=True, stop=True)

                    km_ps = midps.tile([128, NBUX], f32, tag="mmp")
                    mm_big(km_ps, w32["Wk"][:], xmean)
                    nc.scalar.activation(out=tbl[:, :NBUX, 0], in_=km_ps[:],
                                         func=ACTF.Identity, scale=SCALE, bias=bk_s[:])
                    nc.scalar.activation(out=tbl[:, :NBUX, 1], in_=red["kbmax"][:],
                                         func=ACTF.Identity, scale=SCALE, bias=bk_s[:])

                    vm_ps = midps.tile([128, NBUX], f32, tag="mmp")
                    mm_big(vm_ps, w32["Wv"][:], xmean)
                    vmean = midw.tile([128, NBUX], f32, tag="vmean")
                    nc.scalar.activation(out=vmean[:], in_=vm_ps[:],
                                         func=ACTF.Identity, bias=b["bv"][:])
                    vmax = midw.tile([128, NBUX], f32, tag="vmax")
                    nc.scalar.activation(out=vmax[:], in_=red["vbmax"][:],
                                         func=ACTF.Identity, bias=b["bv"][:])
                    vc_ps = midps.tile([128, NBUX], f32, tag="mmp")
                    for j in range(0, NBUX, 512):
                        e = min(j + 512, NBUX)
                        nc.tensor.matmul(out=vc_ps[:, j:e], lhsT=w["Wvca"][:],
                                         rhs=vmean[:, j:e], start=True, stop=False)
                        nc.tensor.matmul(out=vc_ps[:, j:e], lhsT=w["Wvcb"][:],
                                         rhs=vmax[:, j:e], start=False, stop=True)
                    nc.scalar.activation(out=tbl[:, :NBUX, 2], in_=vc_ps[:],
                                         func=ACTF.Identity, bias=b["bvc"][:])
                    nc.vector.memset(tbl[:, NBUX, :], 0.0)

                    CTXB = max(16, (-(-NSLOT // 8) // 16) * 16)
                    for j in range(0, NSLOT, CTXB):
                        e = min(j + CTXB, NSLOT)
                        nc.gpsimd.ap_gather(
                            out_ap=ctxslot[:, j:e, :], in_ap=tbl[:],
                            idxs_ap=sidx_sb[:, j // 16:e // 16], channels=128,
                            num_elems=NBUX + 1, d=3, num_idxs=e - j)

            # ---------------- pass 2 ----------------
            with tc.tile_pool(name="p2w", bufs=3) as p2w, \
                 tc.tile_pool(name="p2ps", bufs=2, space="PSUM") as p2ps, \
                 tc.tile_pool(name="p2po", bufs=2, space="PSUM") as p2po:
                for cj in range(NCHUNK // 2):
                    xT2 = p2w.tile([128, 2 * CHUNK], f32r, tag="xT2")
                    nc.sync.dma_start(out=xT2[:],
                                      in_=xtp[:, cj * 2 * CHUNK:(cj + 1) * 2 * CHUNK])
                    oT2 = p2w.tile([128, 2, CHUNK], f32, tag="oTs")
                    for half in range(2):
                        ci = cj * 2 + half
                        sl0 = ci * 32
                        qT_ps = p2ps.tile([128, CHUNK], f32, tag="qT")
                        nc.tensor.matmul(
                            out=qT_ps[:], lhsT=w["Wq"][:],
                            rhs=xT2[:, half * CHUNK:(half + 1) * CHUNK],
                            start=True, stop=True)
                        ctx = ctxslot[:, sl0:sl0 + 32, :]
                        inter = p2w.tile([128, 2, 32, SLOT], f32r, tag="inter")
                        qv = qT_ps[:].rearrange("p (s e) -> p s e", e=SLOT)
                        for u in range(2):
                            nc.vector.scalar_tensor_tensor(
                                out=inter[:, u], in0=qv, scalar=b["bq"][:],
                                in1=ctx[:, :, u:u + 1].broadcast_to([128, 32, SLOT]),
                                op0=ALU.add, op1=ALU.mult)
                        h1_ps = p2ps.tile([128, CHUNK], f32, tag="h1")
                        nc.tensor.matmul(
                            out=h1_ps[:], lhsT=w["Wg1a"][:],
                            rhs=inter[:, 0].rearrange("p a b -> p (a b)"),
                            start=True, stop=False)
                        nc.tensor.matmul(
                            out=h1_ps[:], lhsT=w["Wg1b"][:],
                            rhs=inter[:, 1].rearrange("p a b -> p (a b)"),
                            start=False, stop=True)
                        h1 = p2w.tile([128, CHUNK], f32r, tag="h1s")
                        nc.scalar.activation(out=h1[:], in_=h1_ps[:], func=ACTF.Relu,
                                             bias=b["bg1"][:])
                        h2_ps = p2ps.tile([128, CHUNK], f32, tag="h2")
                        nc.tensor.matmul(out=h2_ps[:], lhsT=w["Wg2"][:],
                                         rhs=h1[:], start=True, stop=True)
                        gate = p2w.tile([128, CHUNK], f32, tag="gate")
                        nc.scalar.activation(out=gate[:], in_=h2_ps[:],
                                             func=ACTF.Sigmoid, bias=b["bg2"][:])
                        gv = p2w.tile([128, 32, SLOT], f32r, tag="gv")
                        nc.gpsimd.tensor_tensor(
                            out=gv[:],
                            in0=gate[:].rearrange("p (s e) -> p s e", e=SLOT),
                            in1=ctx[:, :, 2:3].broadcast_to([128, 32, SLOT]),
                            op=ALU.mult)
                        oT_ps = p2po.tile([128, CHUNK], f32, tag="oT")
                        nc.tensor.matmul(
                            out=oT_ps[:], lhsT=w["Wp"][:],
                            rhs=gv[:].rearrange("p a b -> p (a b)"),
                            start=True, stop=True)
                        if ci % 2 == 0:
                            nc.vector.tensor_scalar(out=oT2[:, half, :], in0=oT_ps[:],
                                                    scalar1=b["bp"][:], scalar2=None,
                                                    op0=ALU.add)
                        else:
                            nc.scalar.activation(out=oT2[:, half, :], in_=oT_ps[:],
                                                 func=ACTF.Identity, bias=b["bp"][:])
                    nc.gpsimd.dma_start(
                        out=ot[:, cj * 2 * CHUNK:(cj + 1) * 2 * CHUNK], in_=oT2[:])

    with tile.TileContext(nc) as tc:
        for _rep in range(reps):
            _emit(tc)
    nc.finalize()
    return nc


def _make_in_maps(inputs, layout):
    shared = {nm: np.ascontiguousarray(inputs[nm], np.float32)
              for nm in ["Wq", "Wk", "Wv", "Wg1", "Wg2", "Wvc", "Wp",
                         "bq", "bk", "bv", "bg1", "bg2", "bvc", "bp"]}
    x = np.ascontiguousarray(inputs["x"], np.float32)
    in_maps = []
    for core in layout["cores"]:
        perm = core["perm"]
        xp = np.zeros((layout["NPTS"], C), np.float32)
        m = perm >= 0
        xp[m] = x[perm[m]]
        in_maps.append(dict(shared, xp=xp, xtp=np.ascontiguousarray(xp.T),
                            sind=core["slot_ind"], cnt=core["cnt"],
                            bidx=core["bidx"], sidx=core["sidx"]))
    return in_maps


def _assemble_out(results, layout, n):
    out = np.empty((n, C), np.float32)
    for core, r in zip(layout["cores"], results):
        perm = core["perm"]
        m = perm >= 0
        out[perm[m]] = r["ot"].T[m]
    return out


def _run(inputs, layout, trace=False):
    nc = _build_nc(layout)
    in_maps = _make_in_maps(inputs, layout)
    res = bass_utils.run_bass_kernel_spmd(
        nc, in_maps, core_ids=list(range(NCORES)), trace=trace)
    out = _assemble_out(res.results, layout, inputs["x"].shape[0])
    return out, res


def kernel(**inputs):
    ids = np.asarray(inputs["cluster_ids"]).astype(np.int64)
    B = int(inputs["total_buckets"])
    layout = _build_layout(ids, B, NCORES)
    out, _ = _run(inputs, layout, trace=False)
    return out


# ---------------------------------------------------------------------------
# pure-numpy emulation of the device program (for logic validation off-HW)
def kernel_emulate(**inputs):
    ids = np.asarray(inputs["cluster_ids"]).astype(np.int64)
    B = int(inputs["total_buckets"])
    L = _build_layout(ids, B, NCORES)
    NPTS, NBUX, NSLOT, SPB = L["NPTS"], L["NBUX"], L["NSLOT"], L["SPB"]
    x = np.asarray(inputs["x"], np.float32)
    W = {k: np.asarray(inputs[k], np.float32) for k in
         ["Wq", "Wk", "Wv", "Wg1", "Wg2", "Wvc", "Wp",
          "bq", "bk", "bv", "bg1", "bg2", "bvc", "bp"]}
    n = x.shape[0]
    out = np.empty((n, C), np.float32)
    for core in L["cores"]:
        perm = core["perm"]
        m = perm >= 0
        xp = np.zeros((NPTS, C), np.float32)
        xp[m] = x[perm[m]]
        sind = core["slot_ind"].reshape(NPTS, 8)
        # pass 1
        kT = (xp @ W["Wk"]).T  # pre-bias
        vT = (xp @ W["Wv"]).T
        xslot = np.zeros((128, NSLOT + 1), np.float32)
        kslot = np.full((128, NSLOT + 1), NEG_BIG, np.float32)
        vslot = np.full((128, NSLOT + 1), NEG_BIG, np.float32)
        # slot sums via indicator (pads zeroed), slot maxes direct
        ind = np.zeros((NPTS, NSLOT), np.float32)
        srow = np.arange(NPTS) // SLOT
        ind[np.arange(NPTS), srow] = sind[np.arange(NPTS), (np.arange(NPTS) % 128) // 16]
        xslot[:, :NSLOT] = xp.T @ ind
        kslot[:, :NSLOT] = kT.reshape(128, NSLOT, SLOT).max(axis=2)
        vslot[:, :NSLOT] = vT.reshape(128, NSLOT, SLOT).max(axis=2)
        # mid
        def unwrap(arr, n):
            outv = np.zeros(n, np.int64)
            k = np.arange(n)
            outv[k] = arr[(k % 16), k // 16]
            return outv
        bidx = unwrap(core["bidx"], NBUX * SPB)
        sidx = unwrap(core["sidx"], NSLOT)
        g = xslot[:, bidx].reshape(128, NBUX, SPB)
        xbsum = g.sum(axis=2)
        kbmax = kslot[:, bidx].reshape(128, NBUX, SPB).max(axis=2)
        vbmax = vslot[:, bidx].reshape(128, NBUX, SPB).max(axis=2)
        rc = 1.0 / core["cnt"]
        xmean = xbsum * rc
        tbl = np.zeros((128, NBUX + 1, 3), np.float32)
        tbl[:, :NBUX, 0] = SCALE * (W["Wk"].T @ xmean + W["bk"][:, None])
        tbl[:, :NBUX, 1] = SCALE * (kbmax + W["bk"][:, None])
        vmean = W["Wv"].T @ xmean + W["bv"][:, None]
        vmax = vbmax + W["bv"][:, None]
        tbl[:, :NBUX, 2] = (W["Wvc"][:C].T @ vmean + W["Wvc"][C:].T @ vmax
                            + W["bvc"][:, None])
        ctxslot = tbl[:, sidx, :]  # [128, NSLOT, 3]
        # pass 2
        qT = (xp @ W["Wq"]).T + W["bq"][:, None]
        ctxe = np.repeat(ctxslot, SLOT, axis=1)  # [128, NPTS, 3]
        inter1 = qT * ctxe[:, :, 0]
        inter2 = qT * ctxe[:, :, 1]
        h1 = np.maximum(W["Wg1"][:C].T @ inter1 + W["Wg1"][C:].T @ inter2
                        + W["bg1"][:, None], 0.0)
        h2 = W["Wg2"].T @ h1 + W["bg2"][:, None]
        gate = 1.0 / (1.0 + np.exp(-h2))
        gv = gate * ctxe[:, :, 2]
        oT = W["Wp"].T @ gv + W["bp"][:, None]
        out[perm[m]] = oT.T[m]
    return out
